# revision 5
# baseline (speedup 1.0000x reference)
"""Trainium2 Bass kernel for nn_ConvTwist (twisted grouped conv).

Problem: x (32, 512, 56, 56) f32, W (512, 8, 3, 3) f32.
The full 512x512x3x3 kernel is block-diagonal over 16 independent 32-channel
blocks (the group-twist permutation j(i) = i+3 if i%4==0 else i-1 stays inside
blocks of 4 groups = 32 channels). Each block is a dense 32->32 3x3 conv
(with 4 nonzero 8x8 group sub-blocks).

Strategy (per core, data-parallel over batch, 4 images/core):
- Host pre-permutes channels, pads rows to 58 cols, casts to fp16, and builds
  per-tile 32x32 lhsT weight matrices for the 9 kernel offsets.
- Device: conv = 9 shifted matmuls accumulated in PSUM. The PE array is split
  into 16 concurrent 32x32 tiles (tile_position); tile (i,j) handles channel
  block 4i+j: rhs from SBUF partitions 32i (region j), output to PSUM bank i
  partitions 32j. Output channel c = 128*bank + partition, so results land in
  natural channel order for a strided store.
- 56 rows are processed in 7 bands of 8 rows (N=448 <= 512 PSUM bank limit).
- Weight-load path optimization: tile_legalize splits each fp16 matmul into
  (Ldweights, Matmult); the 32-column tile loads serialize on the PE weight
  bus (~34ns each) and dominate. We fuse the pairs back into self-loading
  matmuls and compile walrus with --enable-ldw-opt=true, which elides the
  reload for back-to-back matmuls sharing the same stationary operand (the
  two bands of a pair), halving weight-bus traffic.
- Output is stored as fp16 (halves the store DMA bytes); host casts to f32.
"""
import numpy as np

import concourse.bacc as bacc
import concourse.mybir as mybir
import concourse.tile as tile
from concourse.bass_utils import run_bass_kernel_spmd


def _enable_ldw_opt():
    """Flip walrus's --enable-ldw-opt so back-to-back matmuls sharing the same
    stationary operand reuse the loaded weights instead of reloading."""
    import concourse.bass_utils as bu
    if getattr(bu, "_ldw_opt_patched", False):
        return
    orig = bu.run_command

    def patched(cmd, **kw):
        cmd = [c.replace("--enable-ldw-opt=false", "--enable-ldw-opt=true")
               if isinstance(c, str) else c for c in cmd]
        return orig(cmd, **kw)

    bu.run_command = patched
    bu._ldw_opt_patched = True


def _fuse_ldweights_into_matmuls(nc):
    """Fuse tile_legalize's (InstLdweights, InstMatmult) pairs back into
    self-loading matmuls; --enable-ldw-opt then dedupes repeated loads.
    Run after the TileContext exits, before nc.compile()."""
    n_fused = 0
    for fn in nc.m.functions:
        for blk in fn.blocks:
            insts = list(blk.instructions)
            keep = []
            pending = None
            for ins in insts:
                if ins.opcode == "Ldweights":
                    assert pending is None
                    pending = ins
                    continue
                if ins.opcode == "Matmult" and pending is not None:
                    ins.merge_dependencies_from(pending)
                    ins.ldweights = True
                    pending = None
                    n_fused += 1
                keep.append(ins)
            assert pending is None
            if len(keep) != len(insts):
                blk.instructions[:] = keep
    return n_fused

N_CORES = 8
B = 32               # full batch
BC = B // N_CORES    # images per core
C = 512              # channels
H = W_ = 56          # spatial
WP = 58              # padded row width
HWP = H * WP         # 3248 padded pixels / channel
HW = H * W_          # 3136 pixels / channel
BAND = 8             # rows per band
NB = H // BAND       # 7 bands
NBAND = BAND * W_    # 448 free-dim per band

# offset order: dy=0 first so the start=True matmul covers the full band
OFFS = [(0, -1), (0, 0), (0, 1), (-1, -1), (-1, 0), (-1, 1), (1, -1), (1, 0), (1, 1)]

F16 = mybir.dt.float16
F32 = mybir.dt.float32

_CACHE = {}


def _build_nc():
    _enable_ldw_opt()
    nc = bacc.Bacc(None, target_bir_lowering=False)
    x_d = nc.dram_tensor("x", [BC, 128, 4 * HWP], F16, kind="ExternalInput")
    w_d = nc.dram_tensor("w", [128, 4 * 9 * 32], F16, kind="ExternalInput")
    o_d = nc.dram_tensor("o", [BC, C, HW], F16, kind="ExternalOutput")

    with tile.TileContext(nc) as tc:
        with (
            tc.tile_pool(name="xp", bufs=4) as xpool,
            tc.tile_pool(name="wp", bufs=1) as wpool,
            tc.tile_pool(name="op", bufs=4) as opool,
            tc.tile_pool(name="ps", bufs=2, space="PSUM") as pspool,
        ):
            wt = wpool.tile([128, 4 * 9 * 32], F16, tag="w", name="wt")
            nc.sync.dma_start(out=wt[:], in_=w_d[:])
            # All x loads issued up front (no deps; bufs=4 holds all images).
            # Chunked by band-pair row ranges so early bands' matmuls wait
            # only on their chunk (subtile deps).
            xts = []
            chunks = [(0, 18), (18, 34), (34, 50), (50, 56)]
            for n in range(BC):
                xt = xpool.tile([128, 4 * HWP], F16, tag="x", name=f"xt{n}")
                xts.append(xt)
            for c0, c1 in chunks:
                for n in range(BC):
                    xr = x_d[n].rearrange("p (r y c) -> p r y c", r=4, c=WP)
                    xtr = xts[n].rearrange("p (r y c) -> p r y c", r=4, c=WP)
                    nc.sync.dma_start(out=xtr[:, :, c0:c1], in_=xr[:, :, c0:c1])
            for p in range(BC // 2):
                imgs = (2 * p, 2 * p + 1)
                # per-channel padded image views: [part, region, row, col]
                xvs = {n: xts[n].rearrange("p (r y c) -> p r y c", r=4, c=WP)
                       for n in imgs}
                for b in range(NB):
                    r0 = b * BAND
                    # slot s = image within the pair; both slots share the
                    # same weights per (offset, tile) -> ldw-opt dedupes
                    pst = {s: [pspool.tile([128, NBAND], F32, tag=f"ps{s}{i}",
                                           name=f"ps{p}_{b}_{s}_{i}", bufs=1)
                               for i in range(4)] for s in range(2)}
                    for o_idx, (dy, dx) in enumerate(OFFS):
                        y0 = max(r0, -dy)
                        y1 = min(r0 + BAND, 56 - max(0, dy))
                        nr = y1 - y0
                        po = (y0 - r0) * W_
                        for s, n in enumerate(imgs):
                            for i in range(4):
                                for j in range(4):
                                    lhsT = wt[32 * i:32 * i + 32,
                                              (j * 9 + o_idx) * 32:(j * 9 + o_idx) * 32 + 32]
                                    rhs = xvs[n][32 * i:32 * i + 32, j,
                                                 y0 + dy:y0 + dy + nr, 1 + dx:1 + dx + W_]
                                    nc.tensor.matmul(
                                        pst[s][i][32 * j:32 * j + 32, po:po + nr * W_],
                                        lhsT, rhs,
                                        start=(o_idx == 0), stop=(o_idx == len(OFFS) - 1),
                                        tile_position=(32 * i, 32 * j))
                    for s, n in enumerate(imgs):
                        ot = opool.tile([128, 4 * NBAND], F16, tag="o", name=f"ot{n}_{b}")
                        for i in range(4):
                            # split the evacuation: DVE and ACT read
                            # different PSUM banks in parallel
                            dst_seg = ot[:, i * NBAND:(i + 1) * NBAND]
                            if i < 2:
                                nc.vector.tensor_copy(out=dst_seg, in_=pst[s][i][:, :])
                            else:
                                nc.scalar.copy(out=dst_seg, in_=pst[s][i][:, :])
                        dst = o_d[n].rearrange("(i p) f -> p i f", p=128)[
                            :, :, r0 * W_:r0 * W_ + NBAND]
                        nc.sync.dma_start(out=dst, in_=ot.rearrange("p (i t) -> p i t", i=4))
    _fuse_ldweights_into_matmuls(nc)
    nc.compile()
    return nc


def _prep_weights(W: np.ndarray) -> np.ndarray:
    """W (512, 8, 3, 3) f32 -> (128, 4*9*32) f16 lhsT layout.

    partition p = 32*i + k ; free idx = (j*9 + o)*32 + m
    holds W_blk[4i+j][m, k, dy, dx] for offset o = OFFS[o_idx].
    """
    Wg = W.reshape(64, 8, 8, 3, 3)  # [group gi][oc][ic][dy][dx]
    # block-level dense 32x32 kernels
    Wb = np.zeros((16, 32, 32, 3, 3), dtype=np.float32)  # [b][m(out)][k(in)][dy][dx]
    for gi in range(64):
        b, u = divmod(gi, 4)
        jg = gi + 3 if gi % 4 == 0 else gi - 1  # input group (twist)
        v = jg % 4
        assert jg // 4 == b
        Wb[b, 8 * u:8 * u + 8, 8 * v:8 * v + 8] = Wg[gi]
    out = np.zeros((128, 4 * 9 * 32), dtype=np.float32)
    for i in range(4):
        for j in range(4):
            blk = Wb[4 * i + j]  # [m][k][dy][dx]
            for o_idx, (dy, dx) in enumerate(OFFS):
                # lhsT[k, m]
                out[32 * i:32 * i + 32, (j * 9 + o_idx) * 32:(j * 9 + o_idx) * 32 + 32] = \
                    blk[:, :, dy + 1, dx + 1].T
    return out.astype(np.float16)


def _prep_x(x_shard: np.ndarray) -> np.ndarray:
    """x_shard (BC, 512, 56, 56) f32 -> (BC, 128, 4*HWP) f16 permuted+padded.

    Device partition p = 32*s + k of region r holds original channel
    c = 128*s + 32*r + k (so tile (i,j) reading region j, slice i gets
    block 4i+j), padded to 58 cols.
    """
    n = x_shard.shape[0]
    xs = x_shard.reshape(n, 4, 4, 32, H, W_)          # [n][s][r][k][y][x]
    xs = xs.transpose(0, 1, 3, 2, 4, 5)               # [n][s][k][r][y][x]
    xp = np.zeros((n, 4, 32, 4, H, WP), dtype=np.float16)
    xp[..., 1:57] = xs
    return xp.reshape(n, 128, 4, HWP).reshape(n, 128, 4 * HWP)


def kernel(x: np.ndarray, W: np.ndarray) -> np.ndarray:
    if "nc" not in _CACHE:
        _CACHE["nc"] = _build_nc()
    nc = _CACHE["nc"]

    w_dev = _prep_weights(np.asarray(W, dtype=np.float32))
    x = np.asarray(x, dtype=np.float32)
    in_maps = []
    for c in range(N_CORES):
        shard = x[c * BC:(c + 1) * BC]
        in_maps.append({"x": _prep_x(shard), "w": w_dev})

    res = run_bass_kernel_spmd(nc, in_maps, core_ids=list(range(N_CORES)))
    outs = [res.results[c]["o"].reshape(BC, C, H, W_).astype(np.float32)
            for c in range(N_CORES)]
    return np.concatenate(outs, axis=0)


if __name__ == "__main__":
    # quick self-test against a numpy reference
    rng = np.random.default_rng(0)
    x = rng.standard_normal((B, C, H, W_), dtype=np.float32)
    Wt = (rng.standard_normal((C, 8, 3, 3)) * 0.12).astype(np.float32)
    out = kernel(x, Wt)
    print("out", out.shape, out.dtype)


# revision 7
# speedup vs baseline: 1.0537x; 1.0537x over previous
"""Trainium2 Bass kernel for nn_ConvTwist (twisted grouped conv).

Problem: x (32, 512, 56, 56) f32, W (512, 8, 3, 3) f32.
The full 512x512x3x3 kernel is block-diagonal over 16 independent 32-channel
blocks (the group-twist permutation j(i) = i+3 if i%4==0 else i-1 stays inside
blocks of 4 groups = 32 channels). Each block is a dense 32->32 3x3 conv
(with 4 nonzero 8x8 group sub-blocks).

Strategy (per core, data-parallel over batch, 4 images/core):
- Host pre-permutes channels, pads rows to 58 cols, casts to fp16, and builds
  per-tile 32x32 lhsT weight matrices for the 9 kernel offsets.
- Device: conv = 9 shifted matmuls accumulated in PSUM. The PE array is split
  into 16 concurrent 32x32 tiles (tile_position); tile (i,j) handles channel
  block 4i+j: rhs from SBUF partitions 32i (region j), output to PSUM bank i
  partitions 32j. Output channel c = 128*bank + partition, so results land in
  natural channel order for a strided store.
- 56 rows are processed in 7 bands of 8 rows (N=448 <= 512 PSUM bank limit).
- Weight-load path optimization: tile_legalize splits each fp16 matmul into
  (Ldweights, Matmult); the 32-column tile loads serialize on the PE weight
  bus (~34ns each) and dominate. We fuse the pairs back into self-loading
  matmuls and compile walrus with --enable-ldw-opt=true, which elides the
  reload for back-to-back matmuls sharing the same stationary operand (the
  two bands of a pair), halving weight-bus traffic.
- Output is stored as fp16 (halves the store DMA bytes); host casts to f32.
"""
import numpy as np

import concourse.bacc as bacc
import concourse.mybir as mybir
import concourse.tile as tile
from concourse.bass_utils import run_bass_kernel_spmd


def _enable_ldw_opt():
    """Flip walrus's --enable-ldw-opt so back-to-back matmuls sharing the same
    stationary operand reuse the loaded weights instead of reloading."""
    import concourse.bass_utils as bu
    if getattr(bu, "_ldw_opt_patched", False):
        return
    orig = bu.run_command

    def patched(cmd, **kw):
        cmd = [c.replace("--enable-ldw-opt=false", "--enable-ldw-opt=true")
               if isinstance(c, str) else c for c in cmd]
        return orig(cmd, **kw)

    bu.run_command = patched
    bu._ldw_opt_patched = True


def _fuse_ldweights_into_matmuls(nc):
    """Fuse tile_legalize's (InstLdweights, InstMatmult) pairs back into
    self-loading matmuls; --enable-ldw-opt then dedupes repeated loads.
    Run after the TileContext exits, before nc.compile()."""
    n_fused = 0
    for fn in nc.m.functions:
        for blk in fn.blocks:
            insts = list(blk.instructions)
            keep = []
            pending = None
            for ins in insts:
                if ins.opcode == "Ldweights":
                    assert pending is None
                    pending = ins
                    continue
                if ins.opcode == "Matmult" and pending is not None:
                    ins.merge_dependencies_from(pending)
                    ins.ldweights = True
                    pending = None
                    n_fused += 1
                keep.append(ins)
            assert pending is None
            if len(keep) != len(insts):
                blk.instructions[:] = keep
    return n_fused

N_CORES = 8
B = 32               # full batch
BC = B // N_CORES    # images per core
C = 512              # channels
H = W_ = 56          # spatial
WP = 58              # padded row width
HWP = H * WP         # 3248 padded pixels / channel
HW = H * W_          # 3136 pixels / channel
BAND = 8             # rows per band
NB = H // BAND       # 7 bands
NBAND = BAND * W_    # 448 free-dim per band

# offset order: dy=0 first so the start=True matmul covers the full band
OFFS = [(0, -1), (0, 0), (0, 1), (-1, -1), (-1, 0), (-1, 1), (1, -1), (1, 0), (1, 1)]

F16 = mybir.dt.float16
F32 = mybir.dt.float32

_CACHE = {}


def _build_nc():
    _enable_ldw_opt()
    nc = bacc.Bacc(None, target_bir_lowering=False)
    x_d = nc.dram_tensor("x", [BC, 128, 4 * HWP], F16, kind="ExternalInput")
    w_d = nc.dram_tensor("w", [128, 4 * 9 * 32], F16, kind="ExternalInput")
    o_d = nc.dram_tensor("o", [BC, C, HW], F16, kind="ExternalOutput")

    with tile.TileContext(nc) as tc:
        with (
            tc.tile_pool(name="xp", bufs=4) as xpool,
            tc.tile_pool(name="wp", bufs=1) as wpool,
            tc.tile_pool(name="op", bufs=4) as opool,
            tc.tile_pool(name="ps", bufs=2, space="PSUM") as pspool,
        ):
            wt = wpool.tile([128, 4 * 9 * 32], F16, tag="w", name="wt")
            nc.sync.dma_start(out=wt[:], in_=w_d[:])
            # All x loads issued up front (no deps; bufs=4 holds all images).
            # Chunked by band-pair row ranges so early bands' matmuls wait
            # only on their chunk (subtile deps).
            xts = []
            chunks = [(0, 18), (18, 34), (34, 50), (50, 56)]
            for n in range(BC):
                xt = xpool.tile([128, 4 * HWP], F16, tag="x", name=f"xt{n}")
                xts.append(xt)
            # issue order follows consumption order: all of pair 0's chunks
            # (both images, band-pair by band-pair), then pair 1's
            for p in range(BC // 2):
                for c0, c1 in chunks:
                    for n in (2 * p, 2 * p + 1):
                        xr = x_d[n].rearrange("p (r y c) -> p r y c", r=4, c=WP)
                        xtr = xts[n].rearrange("p (r y c) -> p r y c", r=4, c=WP)
                        nc.sync.dma_start(out=xtr[:, :, c0:c1], in_=xr[:, :, c0:c1])
            for p in range(BC // 2):
                imgs = (2 * p, 2 * p + 1)
                # per-channel padded image views: [part, region, row, col]
                xvs = {n: xts[n].rearrange("p (r y c) -> p r y c", r=4, c=WP)
                       for n in imgs}
                for b in range(NB):
                    r0 = b * BAND
                    # slot s = image within the pair; both slots share the
                    # same weights per (offset, tile) -> ldw-opt dedupes
                    pst = {s: [pspool.tile([128, NBAND], F32, tag=f"ps{s}{i}",
                                           name=f"ps{p}_{b}_{s}_{i}", bufs=1)
                               for i in range(4)] for s in range(2)}
                    for o_idx, (dy, dx) in enumerate(OFFS):
                        y0 = max(r0, -dy)
                        y1 = min(r0 + BAND, 56 - max(0, dy))
                        nr = y1 - y0
                        po = (y0 - r0) * W_
                        for s, n in enumerate(imgs):
                            for i in range(4):
                                for j in range(4):
                                    lhsT = wt[32 * i:32 * i + 32,
                                              (j * 9 + o_idx) * 32:(j * 9 + o_idx) * 32 + 32]
                                    rhs = xvs[n][32 * i:32 * i + 32, j,
                                                 y0 + dy:y0 + dy + nr, 1 + dx:1 + dx + W_]
                                    nc.tensor.matmul(
                                        pst[s][i][32 * j:32 * j + 32, po:po + nr * W_],
                                        lhsT, rhs,
                                        start=(o_idx == 0), stop=(o_idx == len(OFFS) - 1),
                                        tile_position=(32 * i, 32 * j))
                    for s, n in enumerate(imgs):
                        ot = opool.tile([128, 4 * NBAND], F16, tag="o", name=f"ot{n}_{b}")
                        for i in range(4):
                            # split the evacuation by bank parity so the next
                            # unit's first two banks (i=0 on DVE, i=1 on ACT)
                            # free up in parallel
                            dst_seg = ot[:, i * NBAND:(i + 1) * NBAND]
                            if i % 2 == 0:
                                nc.vector.tensor_copy(out=dst_seg, in_=pst[s][i][:, :])
                            else:
                                nc.scalar.copy(out=dst_seg, in_=pst[s][i][:, :])
                        dst = o_d[n].rearrange("(i p) f -> p i f", p=128)[
                            :, :, r0 * W_:r0 * W_ + NBAND]
                        nc.sync.dma_start(out=dst, in_=ot.rearrange("p (i t) -> p i t", i=4))
    _fuse_ldweights_into_matmuls(nc)
    nc.compile()
    return nc


def _prep_weights(W: np.ndarray) -> np.ndarray:
    """W (512, 8, 3, 3) f32 -> (128, 4*9*32) f16 lhsT layout.

    partition p = 32*i + k ; free idx = (j*9 + o)*32 + m
    holds W_blk[4i+j][m, k, dy, dx] for offset o = OFFS[o_idx].
    """
    Wg = W.reshape(64, 8, 8, 3, 3)  # [group gi][oc][ic][dy][dx]
    # block-level dense 32x32 kernels
    Wb = np.zeros((16, 32, 32, 3, 3), dtype=np.float32)  # [b][m(out)][k(in)][dy][dx]
    for gi in range(64):
        b, u = divmod(gi, 4)
        jg = gi + 3 if gi % 4 == 0 else gi - 1  # input group (twist)
        v = jg % 4
        assert jg // 4 == b
        Wb[b, 8 * u:8 * u + 8, 8 * v:8 * v + 8] = Wg[gi]
    out = np.zeros((128, 4 * 9 * 32), dtype=np.float32)
    for i in range(4):
        for j in range(4):
            blk = Wb[4 * i + j]  # [m][k][dy][dx]
            for o_idx, (dy, dx) in enumerate(OFFS):
                # lhsT[k, m]
                out[32 * i:32 * i + 32, (j * 9 + o_idx) * 32:(j * 9 + o_idx) * 32 + 32] = \
                    blk[:, :, dy + 1, dx + 1].T
    return out.astype(np.float16)


def _prep_x(x_shard: np.ndarray) -> np.ndarray:
    """x_shard (BC, 512, 56, 56) f32 -> (BC, 128, 4*HWP) f16 permuted+padded.

    Device partition p = 32*s + k of region r holds original channel
    c = 128*s + 32*r + k (so tile (i,j) reading region j, slice i gets
    block 4i+j), padded to 58 cols.
    """
    n = x_shard.shape[0]
    xs = x_shard.reshape(n, 4, 4, 32, H, W_)          # [n][s][r][k][y][x]
    xs = xs.transpose(0, 1, 3, 2, 4, 5)               # [n][s][k][r][y][x]
    xp = np.zeros((n, 4, 32, 4, H, WP), dtype=np.float16)
    xp[..., 1:57] = xs
    return xp.reshape(n, 128, 4, HWP).reshape(n, 128, 4 * HWP)


def kernel(x: np.ndarray, W: np.ndarray) -> np.ndarray:
    if "nc" not in _CACHE:
        _CACHE["nc"] = _build_nc()
    nc = _CACHE["nc"]

    w_dev = _prep_weights(np.asarray(W, dtype=np.float32))
    x = np.asarray(x, dtype=np.float32)
    in_maps = []
    for c in range(N_CORES):
        shard = x[c * BC:(c + 1) * BC]
        in_maps.append({"x": _prep_x(shard), "w": w_dev})

    res = run_bass_kernel_spmd(nc, in_maps, core_ids=list(range(N_CORES)))
    outs = [res.results[c]["o"].reshape(BC, C, H, W_).astype(np.float32)
            for c in range(N_CORES)]
    return np.concatenate(outs, axis=0)


if __name__ == "__main__":
    # quick self-test against a numpy reference
    rng = np.random.default_rng(0)
    x = rng.standard_normal((B, C, H, W_), dtype=np.float32)
    Wt = (rng.standard_normal((C, 8, 3, 3)) * 0.12).astype(np.float32)
    out = kernel(x, Wt)
    print("out", out.shape, out.dtype)


# revision 9
# speedup vs baseline: 1.0601x; 1.0060x over previous
"""Trainium2 Bass kernel for nn_ConvTwist (twisted grouped conv).

Problem: x (32, 512, 56, 56) f32, W (512, 8, 3, 3) f32.
The full 512x512x3x3 kernel is block-diagonal over 16 independent 32-channel
blocks (the group-twist permutation j(i) = i+3 if i%4==0 else i-1 stays inside
blocks of 4 groups = 32 channels). Each block is a dense 32->32 3x3 conv
(with 4 nonzero 8x8 group sub-blocks).

Strategy (per core, data-parallel over batch, 4 images/core):
- Host pre-permutes channels, pads rows to 58 cols, casts to fp16, and builds
  per-tile 32x32 lhsT weight matrices for the 9 kernel offsets.
- Device: conv = 9 shifted matmuls accumulated in PSUM. The PE array is split
  into 16 concurrent 32x32 tiles (tile_position); tile (i,j) handles channel
  block 4i+j: rhs from SBUF partitions 32i (region j), output to PSUM bank i
  partitions 32j. Output channel c = 128*bank + partition, so results land in
  natural channel order for a strided store.
- 56 rows are processed in 7 bands of 8 rows (N=448 <= 512 PSUM bank limit).
- Weight-load path optimization: tile_legalize splits each fp16 matmul into
  (Ldweights, Matmult); the 32-column tile loads serialize on the PE weight
  bus (~34ns each) and dominate. We fuse the pairs back into self-loading
  matmuls and compile walrus with --enable-ldw-opt=true, which elides the
  reload for back-to-back matmuls sharing the same stationary operand (the
  two bands of a pair), halving weight-bus traffic.
- Output is stored as fp16 (halves the store DMA bytes); host casts to f32.
"""
import numpy as np

import concourse.bacc as bacc
import concourse.mybir as mybir
import concourse.tile as tile
from concourse.bass_utils import run_bass_kernel_spmd


def _enable_ldw_opt():
    """Flip walrus's --enable-ldw-opt so back-to-back matmuls sharing the same
    stationary operand reuse the loaded weights instead of reloading."""
    import concourse.bass_utils as bu
    if getattr(bu, "_ldw_opt_patched", False):
        return
    orig = bu.run_command

    def patched(cmd, **kw):
        cmd = [c.replace("--enable-ldw-opt=false", "--enable-ldw-opt=true")
               if isinstance(c, str) else c for c in cmd]
        return orig(cmd, **kw)

    bu.run_command = patched
    bu._ldw_opt_patched = True


def _fuse_ldweights_into_matmuls(nc):
    """Fuse tile_legalize's (InstLdweights, InstMatmult) pairs back into
    self-loading matmuls; --enable-ldw-opt then dedupes repeated loads.
    Run after the TileContext exits, before nc.compile()."""
    n_fused = 0
    for fn in nc.m.functions:
        for blk in fn.blocks:
            insts = list(blk.instructions)
            keep = []
            pending = None
            for ins in insts:
                if ins.opcode == "Ldweights":
                    assert pending is None
                    pending = ins
                    continue
                if ins.opcode == "Matmult" and pending is not None:
                    ins.merge_dependencies_from(pending)
                    ins.ldweights = True
                    pending = None
                    n_fused += 1
                keep.append(ins)
            assert pending is None
            if len(keep) != len(insts):
                blk.instructions[:] = keep
    return n_fused

N_CORES = 8
B = 32               # full batch
BC = B // N_CORES    # images per core
C = 512              # channels
H = W_ = 56          # spatial
WP = 58              # padded row width
HWP = H * WP         # 3248 padded pixels / channel
HW = H * W_          # 3136 pixels / channel
BAND = 8             # rows per band
NB = H // BAND       # 7 bands
NBAND = BAND * W_    # 448 free-dim per band

# offset order: dy=0 first so the start=True matmul covers the full band
OFFS = [(0, -1), (0, 0), (0, 1), (-1, -1), (-1, 0), (-1, 1), (1, -1), (1, 0), (1, 1)]

F16 = mybir.dt.float16
F32 = mybir.dt.float32

_CACHE = {}


def _build_nc():
    _enable_ldw_opt()
    nc = bacc.Bacc(None, target_bir_lowering=False)
    x_d = nc.dram_tensor("x", [BC, 128, 4 * HWP], F16, kind="ExternalInput")
    w_d = nc.dram_tensor("w", [128, 4 * 9 * 32], F16, kind="ExternalInput")
    o_d = nc.dram_tensor("o", [BC, C, HW], F16, kind="ExternalOutput")

    with tile.TileContext(nc) as tc:
        with (
            tc.tile_pool(name="xp", bufs=4) as xpool,
            tc.tile_pool(name="wp", bufs=1) as wpool,
            tc.tile_pool(name="op", bufs=4) as opool,
            tc.tile_pool(name="ps", bufs=2, space="PSUM") as pspool,
        ):
            wt = wpool.tile([128, 4 * 9 * 32], F16, tag="w", name="wt")
            nc.sync.dma_start(out=wt[:], in_=w_d[:])
            # All x loads issued up front (no deps; bufs=4 holds all images).
            # Chunked by band-pair row ranges so early bands' matmuls wait
            # only on their chunk (subtile deps).
            xts = []
            for n in range(BC):
                xt = xpool.tile([128, 4 * HWP], F16, tag="x", name=f"xt{n}")
                xts.append(xt)
            # issue order follows consumption order. Pair 0 is chunked (small
            # first chunk so band 0 starts fast); pair 1 loads whole images —
            # they arrive long before pair 1's compute begins.
            for p in range(BC // 2):
                chunks = [(0, 10), (10, 34), (34, 56)] if p == 0 else [(0, 56)]
                for c0, c1 in chunks:
                    for n in (2 * p, 2 * p + 1):
                        xr = x_d[n].rearrange("p (r y c) -> p r y c", r=4, c=WP)
                        xtr = xts[n].rearrange("p (r y c) -> p r y c", r=4, c=WP)
                        nc.sync.dma_start(out=xtr[:, :, c0:c1], in_=xr[:, :, c0:c1])
            for p in range(BC // 2):
                imgs = (2 * p, 2 * p + 1)
                # per-channel padded image views: [part, region, row, col]
                xvs = {n: xts[n].rearrange("p (r y c) -> p r y c", r=4, c=WP)
                       for n in imgs}
                for b in range(NB):
                    r0 = b * BAND
                    # slot s = image within the pair; both slots share the
                    # same weights per (offset, tile) -> ldw-opt dedupes
                    pst = {s: [pspool.tile([128, NBAND], F32, tag=f"ps{s}{i}",
                                           name=f"ps{p}_{b}_{s}_{i}", bufs=1)
                               for i in range(4)] for s in range(2)}
                    def emit_slot(s, n, o_idx, dy, dx):
                        y0 = max(r0, -dy)
                        y1 = min(r0 + BAND, 56 - max(0, dy))
                        nr = y1 - y0
                        po = (y0 - r0) * W_
                        for i in range(4):
                            for j in range(4):
                                lhsT = wt[32 * i:32 * i + 32,
                                          (j * 9 + o_idx) * 32:(j * 9 + o_idx) * 32 + 32]
                                rhs = xvs[n][32 * i:32 * i + 32, j,
                                             y0 + dy:y0 + dy + nr, 1 + dx:1 + dx + W_]
                                nc.tensor.matmul(
                                    pst[s][i][32 * j:32 * j + 32, po:po + nr * W_],
                                    lhsT, rhs,
                                    start=(o_idx == 0), stop=(o_idx == len(OFFS) - 1),
                                    tile_position=(32 * i, 32 * j))

                    def emit_evac(s, n):
                        ot = opool.tile([128, 4 * NBAND], F16, tag="o", name=f"ot{n}_{b}")
                        for i in range(4):
                            # split the evacuation by bank parity so the next
                            # unit's first two banks (i=0 on DVE, i=1 on ACT)
                            # free up in parallel
                            dst_seg = ot[:, i * NBAND:(i + 1) * NBAND]
                            if i % 2 == 0:
                                nc.vector.tensor_copy(out=dst_seg, in_=pst[s][i][:, :])
                            else:
                                nc.scalar.copy(out=dst_seg, in_=pst[s][i][:, :])
                        dst = o_d[n].rearrange("(i p) f -> p i f", p=128)[
                            :, :, r0 * W_:r0 * W_ + NBAND]
                        nc.sync.dma_start(out=dst, in_=ot.rearrange("p (i t) -> p i t", i=4))

                    for o_idx, (dy, dx) in enumerate(OFFS[:-1]):
                        for s, n in enumerate(imgs):
                            emit_slot(s, n, o_idx, dy, dx)
                    # last offset: finish slot 0, evacuate it while slot 1's
                    # final matmuls stream — the next unit's first (slot-0)
                    # matmuls then find their banks already free
                    o_idx, (dy, dx) = len(OFFS) - 1, OFFS[-1]
                    emit_slot(0, imgs[0], o_idx, dy, dx)
                    emit_evac(0, imgs[0])
                    emit_slot(1, imgs[1], o_idx, dy, dx)
                    emit_evac(1, imgs[1])
    _fuse_ldweights_into_matmuls(nc)
    nc.compile()
    return nc


def _prep_weights(W: np.ndarray) -> np.ndarray:
    """W (512, 8, 3, 3) f32 -> (128, 4*9*32) f16 lhsT layout.

    partition p = 32*i + k ; free idx = (j*9 + o)*32 + m
    holds W_blk[4i+j][m, k, dy, dx] for offset o = OFFS[o_idx].
    """
    Wg = W.reshape(64, 8, 8, 3, 3)  # [group gi][oc][ic][dy][dx]
    # block-level dense 32x32 kernels
    Wb = np.zeros((16, 32, 32, 3, 3), dtype=np.float32)  # [b][m(out)][k(in)][dy][dx]
    for gi in range(64):
        b, u = divmod(gi, 4)
        jg = gi + 3 if gi % 4 == 0 else gi - 1  # input group (twist)
        v = jg % 4
        assert jg // 4 == b
        Wb[b, 8 * u:8 * u + 8, 8 * v:8 * v + 8] = Wg[gi]
    out = np.zeros((128, 4 * 9 * 32), dtype=np.float32)
    for i in range(4):
        for j in range(4):
            blk = Wb[4 * i + j]  # [m][k][dy][dx]
            for o_idx, (dy, dx) in enumerate(OFFS):
                # lhsT[k, m]
                out[32 * i:32 * i + 32, (j * 9 + o_idx) * 32:(j * 9 + o_idx) * 32 + 32] = \
                    blk[:, :, dy + 1, dx + 1].T
    return out.astype(np.float16)


def _prep_x(x_shard: np.ndarray) -> np.ndarray:
    """x_shard (BC, 512, 56, 56) f32 -> (BC, 128, 4*HWP) f16 permuted+padded.

    Device partition p = 32*s + k of region r holds original channel
    c = 128*s + 32*r + k (so tile (i,j) reading region j, slice i gets
    block 4i+j), padded to 58 cols.
    """
    n = x_shard.shape[0]
    xs = x_shard.reshape(n, 4, 4, 32, H, W_)          # [n][s][r][k][y][x]
    xs = xs.transpose(0, 1, 3, 2, 4, 5)               # [n][s][k][r][y][x]
    xp = np.zeros((n, 4, 32, 4, H, WP), dtype=np.float16)
    xp[..., 1:57] = xs
    return xp.reshape(n, 128, 4, HWP).reshape(n, 128, 4 * HWP)


def kernel(x: np.ndarray, W: np.ndarray) -> np.ndarray:
    if "nc" not in _CACHE:
        _CACHE["nc"] = _build_nc()
    nc = _CACHE["nc"]

    w_dev = _prep_weights(np.asarray(W, dtype=np.float32))
    x = np.asarray(x, dtype=np.float32)
    in_maps = []
    for c in range(N_CORES):
        shard = x[c * BC:(c + 1) * BC]
        in_maps.append({"x": _prep_x(shard), "w": w_dev})

    res = run_bass_kernel_spmd(nc, in_maps, core_ids=list(range(N_CORES)))
    outs = [res.results[c]["o"].reshape(BC, C, H, W_).astype(np.float32)
            for c in range(N_CORES)]
    return np.concatenate(outs, axis=0)


if __name__ == "__main__":
    # quick self-test against a numpy reference
    rng = np.random.default_rng(0)
    x = rng.standard_normal((B, C, H, W_), dtype=np.float32)
    Wt = (rng.standard_normal((C, 8, 3, 3)) * 0.12).astype(np.float32)
    out = kernel(x, Wt)
    print("out", out.shape, out.dtype)


# revision 12
# speedup vs baseline: 1.0891x; 1.0274x over previous
"""Trainium2 Bass kernel for nn_ConvTwist (twisted grouped conv).

Problem: x (32, 512, 56, 56) f32, W (512, 8, 3, 3) f32.
The full 512x512x3x3 kernel is block-diagonal over 16 independent 32-channel
blocks (the group-twist permutation j(i) = i+3 if i%4==0 else i-1 stays inside
blocks of 4 groups = 32 channels). Each block is a dense 32->32 3x3 conv
(with 4 nonzero 8x8 group sub-blocks).

Strategy (per core, data-parallel over batch, 4 images/core):
- Host pre-permutes channels, pads rows to 58 cols, casts to fp16, and builds
  per-tile 32x32 lhsT weight matrices for the 9 kernel offsets.
- Device: conv = 9 shifted matmuls accumulated in PSUM. The PE array is split
  into 16 concurrent 32x32 tiles (tile_position); tile (i,j) handles channel
  block 4i+j: rhs from SBUF partitions 32i (region j), output to PSUM bank i
  partitions 32j. Output channel c = 128*bank + partition, so results land in
  natural channel order for a strided store.
- 56 rows are processed in 7 bands of 8 rows (N=448 <= 512 PSUM bank limit).
- Weight-load path optimization: tile_legalize splits each fp16 matmul into
  (Ldweights, Matmult); the 32-column tile loads serialize on the PE weight
  bus (~34ns each) and dominate. We fuse the pairs back into self-loading
  matmuls and compile walrus with --enable-ldw-opt=true, which elides the
  reload for back-to-back matmuls sharing the same stationary operand (the
  two bands of a pair), halving weight-bus traffic.
- Output is stored as fp16 (halves the store DMA bytes); host casts to f32.
"""
import numpy as np

import concourse.bacc as bacc
import concourse.mybir as mybir
import concourse.tile as tile
from concourse.bass_utils import run_bass_kernel_spmd


def _enable_ldw_opt():
    """Flip walrus's --enable-ldw-opt so back-to-back matmuls sharing the same
    stationary operand reuse the loaded weights instead of reloading."""
    import concourse.bass_utils as bu
    if getattr(bu, "_ldw_opt_patched", False):
        return
    orig = bu.run_command

    def patched(cmd, **kw):
        cmd = [c.replace("--enable-ldw-opt=false", "--enable-ldw-opt=true")
               if isinstance(c, str) else c for c in cmd]
        return orig(cmd, **kw)

    bu.run_command = patched
    bu._ldw_opt_patched = True


def _fuse_ldweights_into_matmuls(nc):
    """Fuse tile_legalize's (InstLdweights, InstMatmult) pairs back into
    self-loading matmuls; --enable-ldw-opt then dedupes repeated loads.
    Run after the TileContext exits, before nc.compile()."""
    n_fused = 0
    for fn in nc.m.functions:
        for blk in fn.blocks:
            insts = list(blk.instructions)
            keep = []
            pending = None
            for ins in insts:
                if ins.opcode == "Ldweights":
                    assert pending is None
                    pending = ins
                    continue
                if ins.opcode == "Matmult" and pending is not None:
                    ins.merge_dependencies_from(pending)
                    ins.ldweights = True
                    pending = None
                    n_fused += 1
                keep.append(ins)
            assert pending is None
            if len(keep) != len(insts):
                blk.instructions[:] = keep
    return n_fused

N_CORES = 8
B = 32               # full batch
BC = B // N_CORES    # images per core
C = 512              # channels
H = W_ = 56          # spatial
WP = 58              # padded row width
HWP = H * WP         # 3248 padded pixels / channel
HW = H * W_          # 3136 pixels / channel
BAND = 8             # rows per band
NB = H // BAND       # 7 bands
NBAND = BAND * W_    # 448 free-dim per band

# offset order: dy=0 first so the start=True matmul covers the full band
OFFS = [(0, -1), (0, 0), (0, 1), (-1, -1), (-1, 0), (-1, 1), (1, -1), (1, 0), (1, 1)]

F16 = mybir.dt.float16
F32 = mybir.dt.float32

_CACHE = {}


def _build_nc():
    _enable_ldw_opt()
    nc = bacc.Bacc(None, target_bir_lowering=False)
    x_d = nc.dram_tensor("x", [BC, 128, 4 * HWP], F16, kind="ExternalInput")
    w_d = nc.dram_tensor("w", [128, 4 * 9 * 32], F16, kind="ExternalInput")
    o_d = nc.dram_tensor("o", [BC, C, HW], F16, kind="ExternalOutput")

    with tile.TileContext(nc) as tc:
        with (
            tc.tile_pool(name="xp", bufs=4) as xpool,
            tc.tile_pool(name="wp", bufs=1) as wpool,
            tc.tile_pool(name="op", bufs=8) as opool,
            tc.tile_pool(name="ps", bufs=2, space="PSUM") as pspool,
        ):
            wt = wpool.tile([128, 4 * 9 * 32], F16, tag="w", name="wt")
            nc.sync.dma_start(out=wt[:], in_=w_d[:])
            # All x loads issued up front (no deps; bufs=4 holds all images).
            # Chunked by band-pair row ranges so early bands' matmuls wait
            # only on their chunk (subtile deps).
            xts = []
            for n in range(BC):
                xt = xpool.tile([128, 4 * HWP], F16, tag="x", name=f"xt{n}")
                xts.append(xt)
            # issue order follows consumption order. Pair 0 is chunked (small
            # first chunk so band 0 starts fast); pair 1 loads whole images —
            # they arrive long before pair 1's compute begins.
            for p in range(BC // 2):
                chunks = [(0, 10), (10, 34), (34, 56)]
                for c0, c1 in chunks:
                    for n in (2 * p, 2 * p + 1):
                        xr = x_d[n].rearrange("p (r y c) -> p r y c", r=4, c=WP)
                        xtr = xts[n].rearrange("p (r y c) -> p r y c", r=4, c=WP)
                        nc.sync.dma_start(out=xtr[:, :, c0:c1], in_=xr[:, :, c0:c1])
            for p in range(BC // 2):
                imgs = (2 * p, 2 * p + 1)
                # per-channel padded image views: [part, region, row, col]
                xvs = {n: xts[n].rearrange("p (r y c) -> p r y c", r=4, c=WP)
                       for n in imgs}
                for b in range(NB):
                    r0 = b * BAND
                    # slot s = image within the pair; both slots share the
                    # same weights per (offset, tile) -> ldw-opt dedupes
                    pst = {s: [pspool.tile([128, NBAND], F32, tag=f"ps{s}{i}",
                                           name=f"ps{p}_{b}_{s}_{i}", bufs=1)
                               for i in range(4)] for s in range(2)}
                    def emit_slot(s, n, o_idx, dy, dx):
                        y0 = max(r0, -dy)
                        y1 = min(r0 + BAND, 56 - max(0, dy))
                        nr = y1 - y0
                        po = (y0 - r0) * W_
                        for i in range(4):
                            for j in range(4):
                                lhsT = wt[32 * i:32 * i + 32,
                                          (j * 9 + o_idx) * 32:(j * 9 + o_idx) * 32 + 32]
                                rhs = xvs[n][32 * i:32 * i + 32, j,
                                             y0 + dy:y0 + dy + nr, 1 + dx:1 + dx + W_]
                                nc.tensor.matmul(
                                    pst[s][i][32 * j:32 * j + 32, po:po + nr * W_],
                                    lhsT, rhs,
                                    start=(o_idx == 0), stop=(o_idx == len(OFFS) - 1),
                                    tile_position=(32 * i, 32 * j))

                    def emit_evac(s, n):
                        ot = opool.tile([128, 4 * NBAND], F16, tag="o", name=f"ot{n}_{b}")
                        for i in range(4):
                            # split the evacuation by bank parity so the next
                            # unit's first two banks (i=0 on DVE, i=1 on ACT)
                            # free up in parallel
                            dst_seg = ot[:, i * NBAND:(i + 1) * NBAND]
                            if i % 2 == 0:
                                nc.vector.tensor_copy(out=dst_seg, in_=pst[s][i][:, :])
                            else:
                                nc.scalar.copy(out=dst_seg, in_=pst[s][i][:, :])
                        dst = o_d[n].rearrange("(i p) f -> p i f", p=128)[
                            :, :, r0 * W_:r0 * W_ + NBAND]
                        # stores go out on the (otherwise idle) GpSimd SWDGE
                        # queue so their waits never block the input loads on
                        # the sync queue
                        nc.gpsimd.dma_start(out=dst,
                                            in_=ot.rearrange("p (i t) -> p i t", i=4))

                    for o_idx, (dy, dx) in enumerate(OFFS[:-1]):
                        for s, n in enumerate(imgs):
                            emit_slot(s, n, o_idx, dy, dx)
                    # last offset: finish slot 0, evacuate it while slot 1's
                    # final matmuls stream — the next unit's first (slot-0)
                    # matmuls then find their banks already free
                    o_idx, (dy, dx) = len(OFFS) - 1, OFFS[-1]
                    emit_slot(0, imgs[0], o_idx, dy, dx)
                    emit_evac(0, imgs[0])
                    emit_slot(1, imgs[1], o_idx, dy, dx)
                    emit_evac(1, imgs[1])
    _fuse_ldweights_into_matmuls(nc)
    nc.compile()
    return nc


def _prep_weights(W: np.ndarray) -> np.ndarray:
    """W (512, 8, 3, 3) f32 -> (128, 4*9*32) f16 lhsT layout.

    partition p = 32*i + k ; free idx = (j*9 + o)*32 + m
    holds W_blk[4i+j][m, k, dy, dx] for offset o = OFFS[o_idx].
    """
    Wg = W.reshape(64, 8, 8, 3, 3)  # [group gi][oc][ic][dy][dx]
    # block-level dense 32x32 kernels
    Wb = np.zeros((16, 32, 32, 3, 3), dtype=np.float32)  # [b][m(out)][k(in)][dy][dx]
    for gi in range(64):
        b, u = divmod(gi, 4)
        jg = gi + 3 if gi % 4 == 0 else gi - 1  # input group (twist)
        v = jg % 4
        assert jg // 4 == b
        Wb[b, 8 * u:8 * u + 8, 8 * v:8 * v + 8] = Wg[gi]
    out = np.zeros((128, 4 * 9 * 32), dtype=np.float32)
    for i in range(4):
        for j in range(4):
            blk = Wb[4 * i + j]  # [m][k][dy][dx]
            for o_idx, (dy, dx) in enumerate(OFFS):
                # lhsT[k, m]
                out[32 * i:32 * i + 32, (j * 9 + o_idx) * 32:(j * 9 + o_idx) * 32 + 32] = \
                    blk[:, :, dy + 1, dx + 1].T
    return out.astype(np.float16)


def _prep_x(x_shard: np.ndarray) -> np.ndarray:
    """x_shard (BC, 512, 56, 56) f32 -> (BC, 128, 4*HWP) f16 permuted+padded.

    Device partition p = 32*s + k of region r holds original channel
    c = 128*s + 32*r + k (so tile (i,j) reading region j, slice i gets
    block 4i+j), padded to 58 cols.
    """
    n = x_shard.shape[0]
    xs = x_shard.reshape(n, 4, 4, 32, H, W_)          # [n][s][r][k][y][x]
    xs = xs.transpose(0, 1, 3, 2, 4, 5)               # [n][s][k][r][y][x]
    xp = np.zeros((n, 4, 32, 4, H, WP), dtype=np.float16)
    xp[..., 1:57] = xs
    return xp.reshape(n, 128, 4, HWP).reshape(n, 128, 4 * HWP)


def kernel(x: np.ndarray, W: np.ndarray) -> np.ndarray:
    if "nc" not in _CACHE:
        _CACHE["nc"] = _build_nc()
    nc = _CACHE["nc"]

    w_dev = _prep_weights(np.asarray(W, dtype=np.float32))
    x = np.asarray(x, dtype=np.float32)
    in_maps = []
    for c in range(N_CORES):
        shard = x[c * BC:(c + 1) * BC]
        in_maps.append({"x": _prep_x(shard), "w": w_dev})

    res = run_bass_kernel_spmd(nc, in_maps, core_ids=list(range(N_CORES)))
    outs = [res.results[c]["o"].reshape(BC, C, H, W_).astype(np.float32)
            for c in range(N_CORES)]
    return np.concatenate(outs, axis=0)


if __name__ == "__main__":
    # quick self-test against a numpy reference
    rng = np.random.default_rng(0)
    x = rng.standard_normal((B, C, H, W_), dtype=np.float32)
    Wt = (rng.standard_normal((C, 8, 3, 3)) * 0.12).astype(np.float32)
    out = kernel(x, Wt)
    print("out", out.shape, out.dtype)
